# revision 17
# baseline (speedup 1.0000x reference)
"""CQAttention (BiDAF-style context-query attention) Trainium2 kernel.

Data-parallel over batch: 32 batches -> 8 cores x 4 batches.

Math (per batch, d=128, Lc=2048, Lq=512):
  S = s0[c] + s1[q] + s2[c,q] + bias,  s2 = (Ct*w_mul) @ Qt^T
  S1 = softmax_q(S + NEG*(1-qm));  S2 = softmax_c(S + NEG*(1-cm))
  A  = S1 @ Qt;  Bm = S1 @ (S2^T @ Ct)
  out = [Ct; A; Ct*A; Ct*Bm]^T  -> [4d, Lc]

Device algebra: s0/bias cancel inside softmax_q, s1/bias cancel inside
softmax_c, so both exp passes are the *plain* exp(s2) in the two layouts,
and the per-row/col factors h[q]=exp(s1+qneg), g[c]=exp(s0+cneg) (host
precomputed) are folded multiplicatively into the matmul weight operands:
  rs[c]   = sum_q h[q] X1[q,c]          (lhsT = h replicated)
  An[d,c] = sum_q (Qt*h)[q,d] X1[q,c]
  cs[q]   = sum_c g[c] X2[c,q]          (lhsT = g column)
  NU[d,q] = sum_c (Ct*g)[c,d] X2[c,q]
  Uch[q,d]= NU^T * h[q]/cs[q]
  Bn[d,c] = sum_q Uch[q,d] X1[q,c]
  A = An/rs, Bm = Bn/rs  (PSUM/PSUM divide on DVE)
"""

import sys

sys.path.insert(0, "/opt/trn_rl_repo")

import numpy as np
from contextlib import ExitStack

NEG = -1e30
N_CORES = 8
B_LOC = 4  # batches per core
D = 128
LC = 2048
LQ = 512
NQT = LQ // 128  # 4 q tiles
NCT = LC // 128  # 16 c tiles
NCC = LC // 512  # 4 c chunks
NCG = NCT // 4  # 4 c-tile groups of 4

_NC_CACHE = {}


def _build_bass():
    import concourse.bass as bass
    import concourse.bacc as bacc
    import concourse.tile as tile
    from concourse import mybir, masks

    f32 = mybir.dt.float32
    bf16 = mybir.dt.bfloat16
    Exp = mybir.ActivationFunctionType.Exp
    Alu = mybir.AluOpType

    nc = bacc.Bacc("TRN2", target_bir_lowering=False, debug=False)

    Cin = nc.dram_tensor("C", [B_LOC, D, LC], f32, kind="ExternalInput").ap()
    Qth_in = nc.dram_tensor("Qth", [B_LOC, 128, LQ], bf16, kind="ExternalInput").ap()
    Qwbf_in = nc.dram_tensor("Qwbf", [B_LOC, D, LQ], bf16, kind="ExternalInput").ap()
    Hrep_in = nc.dram_tensor("Hrep", [B_LOC, 128, LQ], bf16, kind="ExternalInput").ap()
    Gcolb_in = nc.dram_tensor("Gcolb", [B_LOC, 128, NCT], bf16, kind="ExternalInput").ap()
    Gcolf_in = nc.dram_tensor("Gcolf", [B_LOC, 128, NCT], f32, kind="ExternalInput").ap()
    Hcolf_in = nc.dram_tensor("Hcolf", [B_LOC, 128, NQT], f32, kind="ExternalInput").ap()
    Out = nc.dram_tensor("out", [B_LOC, 4 * D, LC], f32, kind="ExternalOutput").ap()
    CsScratch = nc.dram_tensor("cs_scratch", [B_LOC, LQ], f32).ap()

    with tile.TileContext(nc) as tc, ExitStack() as ctx:
        cpool = ctx.enter_context(tc.tile_pool(name="const", bufs=1))
        inp = ctx.enter_context(tc.tile_pool(name="inp", bufs=2))
        work = ctx.enter_context(tc.tile_pool(name="work", bufs=2))
        epool = ctx.enter_context(tc.tile_pool(name="epool", bufs=8))
        opool = ctx.enter_context(tc.tile_pool(name="ostg", bufs=8))
        ppw = ctx.enter_context(tc.tile_pool(name="ppw", bufs=2, space="PSUM"))
        pps = ctx.enter_context(tc.tile_pool(name="pps", bufs=4, space="PSUM"))

        ident = cpool.tile([128, 128], bf16, tag="ident")
        masks.make_identity(nc, ident[:])

        for b in range(B_LOC):
            # ---- inputs (small matmul operands first) ----
            qwbf = inp.tile([128, LQ], bf16, tag="qwbf")
            nc.sync.dma_start(qwbf[:], Qwbf_in[b])
            qth = inp.tile([128, LQ], bf16, tag="qth")
            nc.sync.dma_start(qth[:], Qth_in[b])
            cb = inp.tile([128, LC], f32, tag="cb")
            for cc in range(NCC):
                nc.sync.dma_start(
                    cb[:, cc * 512:(cc + 1) * 512],
                    Cin[b][:, cc * 512:(cc + 1) * 512])
            hrep = inp.tile([128, LQ], bf16, tag="hrep")
            nc.sync.dma_start(hrep[:], Hrep_in[b])
            gcolb = inp.tile([128, NCT], bf16, tag="gcolb")
            nc.sync.dma_start(gcolb[:], Gcolb_in[b])
            gcolf = inp.tile([128, NCT], f32, tag="gcolf")
            nc.sync.dma_start(gcolf[:], Gcolf_in[b])
            hcolf = inp.tile([128, NQT], f32, tag="hcolf")
            nc.sync.dma_start(hcolf[:], Hcolf_in[b])

            # warm small DMA'd tensors through DVE so downstream DVE ops
            # carry same-engine deps only (codegen sync-wait slot limits)
            wgcolf = work.tile([128, NCT], f32, tag="wgcolf")
            nc.vector.tensor_copy(wgcolf[:], gcolf[:])
            whcolf = work.tile([128, NQT], f32, tag="whcolf")
            nc.vector.tensor_copy(whcolf[:], hcolf[:])

            # bf16 cast of C on gpsimd (otherwise idle)
            cbf = work.tile([128, LC], bf16, tag="cbf")
            for cc in range(NCC):
                nc.gpsimd.tensor_copy(
                    cbf[:, cc * 512:(cc + 1) * 512],
                    cb[:, cc * 512:(cc + 1) * 512])

            # ---- pass 1: X1[q,c] = exp(s2^T), 4 q-tiles of [128, LC] ----
            e1 = []
            for qt in range(NQT):
                e = epool.tile([128, LC], bf16, tag="e1")
                for h in range(2):
                    ps = ppw.tile([128, LC // 2], f32, tag="wide")
                    for cc in range(2):
                        c0 = (h * 2 + cc) * 512
                        nc.tensor.matmul(
                            ps[:, cc * 512:(cc + 1) * 512],
                            qwbf[:, qt * 128:(qt + 1) * 128],
                            cbf[:, c0:c0 + 512],
                            start=True, stop=True,
                        )
                    nc.scalar.activation(
                        e[:, h * 1024:(h + 1) * 1024], ps[:], Exp)
                e1.append(e)

            # ---- pass 2: X2[c,q] = exp(s2), 4 groups of 4 c-tiles ----
            e2 = []
            for cg in range(NCG):
                e = epool.tile([128, LC], bf16, tag="e2")
                for h in range(2):
                    ps = ppw.tile([128, LC // 2], f32, tag="wide")
                    for j in range(2):
                        ct = cg * 4 + h * 2 + j
                        nc.tensor.matmul(
                            ps[:, j * 512:(j + 1) * 512],
                            cbf[:, ct * 128:(ct + 1) * 128],
                            qwbf[:],
                            start=True, stop=True,
                        )
                    nc.scalar.activation(
                        e[:, h * 1024:(h + 1) * 1024], ps[:], Exp)
                e2.append(e)

            # ---- Ct*g tiles: transpose C then scale by g per c-tile ----
            ctg = []
            for cg in range(NCG):
                ps = pps.tile([128, 512], bf16, tag="sm")
                for j in range(4):
                    ct = cg * 4 + j
                    nc.tensor.transpose(
                        ps[:, j * 128:(j + 1) * 128],
                        cbf[:, ct * 128:(ct + 1) * 128],
                        ident[:],
                    )
                t = work.tile([128, 512], bf16, tag="ctg")
                for j in range(4):
                    ct = cg * 4 + j
                    nc.vector.tensor_scalar_mul(
                        t[:, j * 128:(j + 1) * 128],
                        ps[:, j * 128:(j + 1) * 128],
                        wgcolf[:, ct:ct + 1],
                    )
                ctg.append(t)

            # ---- cs[q] = sum_c g[c] X2[c,q]  (M=1 reduce) ----
            ps_cs = pps.tile([1, 512], f32, tag="sm")
            for cg in range(NCG):
                for j in range(4):
                    ct = cg * 4 + j
                    nc.tensor.matmul(
                        ps_cs[:],
                        gcolb[:, ct:ct + 1],
                        e2[cg][:, j * 512:(j + 1) * 512],
                        start=(ct == 0), stop=(ct == NCT - 1),
                    )
            # copy cs row to SBUF, scatter [1,512] -> [128,4], hc = h/cs
            cs_row = work.tile([1, 512], f32, tag="csrow")
            nc.vector.tensor_copy(cs_row[:], ps_cs[:])
            nc.sync.dma_start(CsScratch[b], cs_row[0:1, :])
            cs_col = work.tile([128, NQT], f32, tag="cscol")
            nc.sync.dma_start(
                cs_col[:], CsScratch[b].rearrange("(j p) -> p j", j=NQT, p=128)
            )
            csr = work.tile([128, NQT], f32, tag="csr")
            nc.vector.reciprocal(csr[:], cs_col[:])
            hc = work.tile([128, NQT], f32, tag="hc")
            nc.vector.tensor_mul(hc[:], csr[:], whcolf[:])

            # ---- NU[d,q] = sum_c (Ct*g)[c,d] X2[c,q] ----
            ps_ut = pps.tile([128, 512], f32, tag="sm")
            for cg in range(NCG):
                for j in range(4):
                    ct = cg * 4 + j
                    nc.tensor.matmul(
                        ps_ut[:],
                        ctg[cg][:, j * 128:(j + 1) * 128],
                        e2[cg][:, j * 512:(j + 1) * 512],
                        start=(ct == 0), stop=(ct == NCT - 1),
                    )
            utb = work.tile([128, 512], bf16, tag="utb")
            nc.vector.tensor_copy(utb[:], ps_ut[:])

            # ---- Uch[q,d] = NU^T * h/cs ----
            ps_u2 = pps.tile([128, 512], bf16, tag="sm")
            for qt in range(NQT):
                nc.tensor.transpose(
                    ps_u2[:, qt * 128:(qt + 1) * 128],
                    utb[:, qt * 128:(qt + 1) * 128],
                    ident[:],
                )
            uch = work.tile([128, 512], bf16, tag="uch")
            for qt in range(NQT):
                nc.vector.tensor_scalar_mul(
                    uch[:, qt * 128:(qt + 1) * 128],
                    ps_u2[:, qt * 128:(qt + 1) * 128],
                    hc[:, qt:qt + 1],
                )

            # ---- rs[c] = sum_q h[q] X1[q,c] (replicated rows); transient
            # psum per c-chunk, immediately reciprocal'd into SBUF ----
            rrec = work.tile([128, LC], f32, tag="rrec")
            for cc in range(NCC):
                ps_rs = pps.tile([128, 512], f32, tag="sm")
                for qt in range(NQT):
                    nc.tensor.matmul(
                        ps_rs[:],
                        hrep[:, qt * 128:(qt + 1) * 128],
                        e1[qt][:, cc * 512:(cc + 1) * 512],
                        start=(qt == 0), stop=(qt == NQT - 1),
                    )
                nc.vector.reciprocal(rrec[:, cc * 512:(cc + 1) * 512], ps_rs[:])

            # ---- An, Bn per c-chunk; outputs ----
            for cc in range(NCC):
                sl = slice(cc * 512, (cc + 1) * 512)

                ps_an = pps.tile([128, 512], f32, tag="sm")
                for qt in range(NQT):
                    nc.tensor.matmul(
                        ps_an[:],
                        qth[:, qt * 128:(qt + 1) * 128],
                        e1[qt][:, sl],
                        start=(qt == 0), stop=(qt == NQT - 1),
                    )
                a_t = opool.tile([128, 512], f32, tag="a")
                nc.vector.scalar_tensor_tensor(
                    a_t[:], ps_an[:], 0.0, rrec[:, sl],
                    op0=Alu.bypass, op1=Alu.mult,
                )

                ps_bn = pps.tile([128, 512], f32, tag="sm")
                for qt in range(NQT):
                    nc.tensor.matmul(
                        ps_bn[:],
                        uch[:, qt * 128:(qt + 1) * 128],
                        e1[qt][:, sl],
                        start=(qt == 0), stop=(qt == NQT - 1),
                    )
                bm_t = opool.tile([128, 512], f32, tag="bm")
                nc.vector.scalar_tensor_tensor(
                    bm_t[:], ps_bn[:], 0.0, rrec[:, sl],
                    op0=Alu.bypass, op1=Alu.mult,
                )

                cta = opool.tile([128, 512], f32, tag="cta")
                nc.gpsimd.tensor_mul(cta[:], cb[:, sl], a_t[:])
                ctb = opool.tile([128, 512], f32, tag="ctb")
                nc.gpsimd.tensor_mul(ctb[:], cb[:, sl], bm_t[:])

                nc.sync.dma_start(Out[b, 0:128, sl], cb[:, sl])
                nc.sync.dma_start(Out[b, 128:256, sl], a_t[:])
                nc.sync.dma_start(Out[b, 256:384, sl], cta[:])
                nc.sync.dma_start(Out[b, 384:512, sl], ctb[:])

    nc.compile()
    return nc


def _prep_inputs(C, Q, Cmask, Qmask, w_c, w_q, w_mul, bias):
    """Host-side precompute of the folded factors; returns per-core in_maps."""
    import ml_dtypes

    C = np.asarray(C, dtype=np.float32)
    Q = np.asarray(Q, dtype=np.float32)
    cm = np.asarray(Cmask, dtype=np.float32)
    qm = np.asarray(Qmask, dtype=np.float32)
    w_c = np.asarray(w_c, dtype=np.float32).reshape(D)
    w_q = np.asarray(w_q, dtype=np.float32).reshape(D)
    w_mul = np.asarray(w_mul, dtype=np.float32).reshape(D)

    B = C.shape[0]
    s0 = np.einsum("bdc,d->bc", C, w_c)  # [B, Lc]
    s1 = np.einsum("bdq,d->bq", Q, w_q)  # [B, Lq]
    # h[q] = exp(s1 + NEG*(1-qm)); g[c] = exp(s0 + NEG*(1-cm))
    h = np.exp(np.where(qm > 0, s1, NEG))  # [B, Lq]
    g = np.exp(np.where(cm > 0, s0, NEG))  # [B, Lc]

    Qw = Q * w_mul[None, :, None]
    bf = ml_dtypes.bfloat16

    in_maps = []
    for core in range(N_CORES):
        sl = slice(core * B_LOC, (core + 1) * B_LOC)
        hb = h[sl]  # [4, Lq]
        gb = g[sl]  # [4, Lc]
        # hrep[b, p, qt*128+k] = h[b, qt*128+p]
        hrep = np.repeat(
            hb.reshape(B_LOC, NQT, 128).transpose(0, 2, 1), 128, axis=2
        ).reshape(B_LOC, 128, LQ)
        # qth[b, p, qt*128+dd] = Q[b, dd, qt*128+p] * h[b, qt*128+p]
        Qs = Q[sl] * hb[:, None, :]  # [4, d, Lq]
        qth = Qs.reshape(B_LOC, D, NQT, 128).transpose(0, 3, 2, 1).reshape(B_LOC, 128, LQ)
        gcol = gb.reshape(B_LOC, NCT, 128).transpose(0, 2, 1)  # [4,128,16]
        hcol = hb.reshape(B_LOC, NQT, 128).transpose(0, 2, 1)  # [4,128,4]
        in_maps.append({
            "C": np.ascontiguousarray(C[sl]),
            "Qth": np.ascontiguousarray(qth).astype(bf),
            "Qwbf": np.ascontiguousarray(Qw[sl]).astype(bf),
            "Hrep": np.ascontiguousarray(hrep).astype(bf),
            "Gcolb": np.ascontiguousarray(gcol).astype(bf),
            "Gcolf": np.ascontiguousarray(gcol),
            "Hcolf": np.ascontiguousarray(hcol),
        })
    return in_maps


def kernel(C, Q, Cmask, Qmask, w_c, w_q, w_mul, bias):
    from concourse.bass_utils import run_bass_kernel_spmd

    if "nc" not in _NC_CACHE:
        _NC_CACHE["nc"] = _build_bass()
    nc = _NC_CACHE["nc"]

    in_maps = _prep_inputs(C, Q, Cmask, Qmask, w_c, w_q, w_mul, bias)
    res = run_bass_kernel_spmd(nc, in_maps, list(range(N_CORES)))
    out = np.concatenate(
        [res.results[i]["out"] for i in range(N_CORES)], axis=0
    ).astype(np.float32)
    return out


# revision 21
# speedup vs baseline: 1.0261x; 1.0261x over previous
"""CQAttention (BiDAF-style context-query attention) Trainium2 kernel.

Data-parallel over batch: 32 batches -> 8 cores x 4 batches.

Math (per batch, d=128, Lc=2048, Lq=512):
  S = s0[c] + s1[q] + s2[c,q] + bias,  s2 = (Ct*w_mul) @ Qt^T
  S1 = softmax_q(S + NEG*(1-qm));  S2 = softmax_c(S + NEG*(1-cm))
  A  = S1 @ Qt;  Bm = S1 @ (S2^T @ Ct)
  out = [Ct; A; Ct*A; Ct*Bm]^T  -> [4d, Lc]

Device algebra: s0/bias cancel inside softmax_q, s1/bias cancel inside
softmax_c, so both exp passes are the *plain* exp(s2) in the two layouts,
and the per-row/col factors h[q]=exp(s1+qneg), g[c]=exp(s0+cneg) (host
precomputed) are folded multiplicatively into the matmul weight operands:
  rs[c]   = sum_q h[q] X1[q,c]          (lhsT = h replicated)
  An[d,c] = sum_q (Qt*h)[q,d] X1[q,c]
  cs[q]   = sum_c g[c] X2[c,q]          (lhsT = g column)
  NU[d,q] = sum_c (Ct*g)[c,d] X2[c,q]
  Uch[q,d]= NU^T * h[q]/cs[q]
  Bn[d,c] = sum_q Uch[q,d] X1[q,c]
  A = An/rs, Bm = Bn/rs  (PSUM/PSUM divide on DVE)
"""

import sys

sys.path.insert(0, "/opt/trn_rl_repo")

import numpy as np
from contextlib import ExitStack

NEG = -1e30
N_CORES = 8
B_LOC = 4  # batches per core
D = 128
LC = 2048
LQ = 512
NQT = LQ // 128  # 4 q tiles
NCT = LC // 128  # 16 c tiles
NCC = LC // 512  # 4 c chunks
NCG = NCT // 4  # 4 c-tile groups of 4

_NC_CACHE = {}


def _build_bass():
    import concourse.bass as bass
    import concourse.bacc as bacc
    import concourse.tile as tile
    from concourse import mybir, masks

    f32 = mybir.dt.float32
    bf16 = mybir.dt.bfloat16
    Exp = mybir.ActivationFunctionType.Exp
    Alu = mybir.AluOpType

    nc = bacc.Bacc("TRN2", target_bir_lowering=False, debug=False)

    Cin = nc.dram_tensor("C", [B_LOC, D, LC], f32, kind="ExternalInput").ap()
    Qth_in = nc.dram_tensor("Qth", [B_LOC, 128, LQ], bf16, kind="ExternalInput").ap()
    Qwbf_in = nc.dram_tensor("Qwbf", [B_LOC, D, LQ], bf16, kind="ExternalInput").ap()
    Hrep_in = nc.dram_tensor("Hrep", [B_LOC, 128, LQ], bf16, kind="ExternalInput").ap()
    Gcolb_in = nc.dram_tensor("Gcolb", [B_LOC, 128, NCT], bf16, kind="ExternalInput").ap()
    Gcolf_in = nc.dram_tensor("Gcolf", [B_LOC, 128, NCT], f32, kind="ExternalInput").ap()
    Hcolf_in = nc.dram_tensor("Hcolf", [B_LOC, 128, NQT], f32, kind="ExternalInput").ap()
    Out = nc.dram_tensor("out", [B_LOC, 4 * D, LC], f32, kind="ExternalOutput").ap()
    CsScratch = nc.dram_tensor("cs_scratch", [B_LOC, LQ], f32).ap()

    with tile.TileContext(nc) as tc, ExitStack() as ctx:
        cpool = ctx.enter_context(tc.tile_pool(name="const", bufs=1))
        inp = ctx.enter_context(tc.tile_pool(name="inp", bufs=2))
        work = ctx.enter_context(tc.tile_pool(name="work", bufs=2))
        epool = ctx.enter_context(tc.tile_pool(name="epool", bufs=8))
        opool = ctx.enter_context(tc.tile_pool(name="ostg", bufs=8))
        ctgpool = ctx.enter_context(tc.tile_pool(name="ctgp", bufs=6))
        ppw = ctx.enter_context(tc.tile_pool(name="ppw", bufs=3, space="PSUM"))
        pps = ctx.enter_context(tc.tile_pool(name="pps", bufs=2, space="PSUM"))

        ident = cpool.tile([128, 128], bf16, tag="ident")
        masks.make_identity(nc, ident[:])

        for b in range(B_LOC):
            # ---- inputs (small matmul operands first) ----
            qwbf = inp.tile([128, LQ], bf16, tag="qwbf")
            nc.sync.dma_start(qwbf[:], Qwbf_in[b])
            qth = inp.tile([128, LQ], bf16, tag="qth")
            nc.sync.dma_start(qth[:], Qth_in[b])
            cb = inp.tile([128, LC], f32, tag="cb")
            for cc in range(NCC):
                nc.sync.dma_start(
                    cb[:, cc * 512:(cc + 1) * 512],
                    Cin[b][:, cc * 512:(cc + 1) * 512])
            hrep = inp.tile([128, LQ], bf16, tag="hrep")
            nc.sync.dma_start(hrep[:], Hrep_in[b])
            gcolb = inp.tile([128, NCT], bf16, tag="gcolb")
            nc.sync.dma_start(gcolb[:], Gcolb_in[b])
            gcolf = inp.tile([128, NCT], f32, tag="gcolf")
            nc.sync.dma_start(gcolf[:], Gcolf_in[b])
            hcolf = inp.tile([128, NQT], f32, tag="hcolf")
            nc.sync.dma_start(hcolf[:], Hcolf_in[b])

            # warm small DMA'd tensors through DVE so downstream DVE ops
            # carry same-engine deps only (codegen sync-wait slot limits)
            wgcolf = work.tile([128, NCT], f32, tag="wgcolf")
            nc.vector.tensor_copy(wgcolf[:], gcolf[:])
            whcolf = work.tile([128, NQT], f32, tag="whcolf")
            nc.vector.tensor_copy(whcolf[:], hcolf[:])

            # bf16 cast of C on gpsimd (otherwise idle)
            cbf = work.tile([128, LC], bf16, tag="cbf")
            for cc in range(NCC):
                nc.gpsimd.tensor_copy(
                    cbf[:, cc * 512:(cc + 1) * 512],
                    cb[:, cc * 512:(cc + 1) * 512])

            # ---- pass 1: X1[q,c] = exp(s2^T), 4 q-tiles of [128, LC] ----
            e1 = []
            for qt in range(NQT):
                e = epool.tile([128, LC], bf16, tag="e1")
                for h in range(2):
                    ps = ppw.tile([128, LC // 2], f32, tag="wide")
                    for cc in range(2):
                        c0 = (h * 2 + cc) * 512
                        nc.tensor.matmul(
                            ps[:, cc * 512:(cc + 1) * 512],
                            qwbf[:, qt * 128:(qt + 1) * 128],
                            cbf[:, c0:c0 + 512],
                            start=True, stop=True,
                        )
                    nc.scalar.activation(
                        e[:, h * 1024:(h + 1) * 1024], ps[:], Exp)
                e1.append(e)

            # ---- pass 2: X2[c,q] = exp(s2), 4 groups of 4 c-tiles ----
            e2 = []
            for cg in range(NCG):
                e = epool.tile([128, LC], bf16, tag="e2")
                for h in range(2):
                    ps = ppw.tile([128, LC // 2], f32, tag="wide")
                    for j in range(2):
                        ct = cg * 4 + h * 2 + j
                        nc.tensor.matmul(
                            ps[:, j * 512:(j + 1) * 512],
                            cbf[:, ct * 128:(ct + 1) * 128],
                            qwbf[:],
                            start=True, stop=True,
                        )
                    nc.scalar.activation(
                        e[:, h * 1024:(h + 1) * 1024], ps[:], Exp)
                e2.append(e)

            # ---- Ct*g tiles: transpose C then scale by g per c-tile ----
            ctg = []
            for cg in range(NCG):
                ps = pps.tile([128, 512], bf16, tag="sm")
                for j in range(4):
                    ct = cg * 4 + j
                    nc.tensor.transpose(
                        ps[:, j * 128:(j + 1) * 128],
                        cbf[:, ct * 128:(ct + 1) * 128],
                        ident[:],
                    )
                t = ctgpool.tile([128, 512], bf16, tag="ctg")
                for j in range(4):
                    ct = cg * 4 + j
                    nc.vector.tensor_scalar_mul(
                        t[:, j * 128:(j + 1) * 128],
                        ps[:, j * 128:(j + 1) * 128],
                        wgcolf[:, ct:ct + 1],
                    )
                ctg.append(t)

            # ---- cs[q] = sum_c g[c] X2[c,q]  (M=1 reduce) ----
            ps_cs = pps.tile([1, 512], f32, tag="sm")
            for cg in range(NCG):
                for j in range(4):
                    ct = cg * 4 + j
                    nc.tensor.matmul(
                        ps_cs[:],
                        gcolb[:, ct:ct + 1],
                        e2[cg][:, j * 512:(j + 1) * 512],
                        start=(ct == 0), stop=(ct == NCT - 1),
                    )
            # copy cs row to SBUF, scatter [1,512] -> [128,4], hc = h/cs
            cs_row = work.tile([1, 512], f32, tag="csrow")
            nc.vector.tensor_copy(cs_row[:], ps_cs[:])
            nc.sync.dma_start(CsScratch[b], cs_row[0:1, :])
            cs_col = work.tile([128, NQT], f32, tag="cscol")
            nc.sync.dma_start(
                cs_col[:], CsScratch[b].rearrange("(j p) -> p j", j=NQT, p=128)
            )
            csr = work.tile([128, NQT], f32, tag="csr")
            nc.vector.reciprocal(csr[:], cs_col[:])
            hc = work.tile([128, NQT], f32, tag="hc")
            nc.vector.tensor_mul(hc[:], csr[:], whcolf[:])

            # ---- NU[d,q] = sum_c (Ct*g)[c,d] X2[c,q] ----
            ps_ut = pps.tile([128, 512], f32, tag="sm")
            for cg in range(NCG):
                for j in range(4):
                    ct = cg * 4 + j
                    nc.tensor.matmul(
                        ps_ut[:],
                        ctg[cg][:, j * 128:(j + 1) * 128],
                        e2[cg][:, j * 512:(j + 1) * 512],
                        start=(ct == 0), stop=(ct == NCT - 1),
                    )
            utb = work.tile([128, 512], bf16, tag="utb")
            nc.vector.tensor_copy(utb[:], ps_ut[:])

            # ---- Uch[q,d] = NU^T * h/cs ----
            ps_u2 = pps.tile([128, 512], bf16, tag="sm")
            for qt in range(NQT):
                nc.tensor.transpose(
                    ps_u2[:, qt * 128:(qt + 1) * 128],
                    utb[:, qt * 128:(qt + 1) * 128],
                    ident[:],
                )
            uch = work.tile([128, 512], bf16, tag="uch")
            for qt in range(NQT):
                nc.vector.tensor_scalar_mul(
                    uch[:, qt * 128:(qt + 1) * 128],
                    ps_u2[:, qt * 128:(qt + 1) * 128],
                    hc[:, qt:qt + 1],
                )

            # ---- rs[c] = sum_q h[q] X1[q,c] (replicated rows); transient
            # psum per c-chunk, immediately reciprocal'd into SBUF ----
            rrec = work.tile([128, LC], f32, tag="rrec")
            for cc in range(NCC):
                ps_rs = pps.tile([128, 512], f32, tag="sm")
                for qt in range(NQT):
                    nc.tensor.matmul(
                        ps_rs[:],
                        hrep[:, qt * 128:(qt + 1) * 128],
                        e1[qt][:, cc * 512:(cc + 1) * 512],
                        start=(qt == 0), stop=(qt == NQT - 1),
                    )
                nc.vector.reciprocal(rrec[:, cc * 512:(cc + 1) * 512], ps_rs[:])

            # ---- An, Bn per c-chunk; outputs ----
            for cc in range(NCC):
                sl = slice(cc * 512, (cc + 1) * 512)

                ps_an = pps.tile([128, 512], f32, tag="sm")
                for qt in range(NQT):
                    nc.tensor.matmul(
                        ps_an[:],
                        qth[:, qt * 128:(qt + 1) * 128],
                        e1[qt][:, sl],
                        start=(qt == 0), stop=(qt == NQT - 1),
                    )
                a_t = opool.tile([128, 512], f32, tag="a")
                nc.vector.scalar_tensor_tensor(
                    a_t[:], ps_an[:], 0.0, rrec[:, sl],
                    op0=Alu.bypass, op1=Alu.mult,
                )

                ps_bn = pps.tile([128, 512], f32, tag="sm")
                for qt in range(NQT):
                    nc.tensor.matmul(
                        ps_bn[:],
                        uch[:, qt * 128:(qt + 1) * 128],
                        e1[qt][:, sl],
                        start=(qt == 0), stop=(qt == NQT - 1),
                    )
                bm_t = opool.tile([128, 512], f32, tag="bm")
                nc.vector.scalar_tensor_tensor(
                    bm_t[:], ps_bn[:], 0.0, rrec[:, sl],
                    op0=Alu.bypass, op1=Alu.mult,
                )

                cta = opool.tile([128, 512], f32, tag="cta")
                nc.gpsimd.tensor_mul(cta[:], cb[:, sl], a_t[:])
                ctb = opool.tile([128, 512], f32, tag="ctb")
                nc.gpsimd.tensor_mul(ctb[:], cb[:, sl], bm_t[:])

                nc.sync.dma_start(Out[b, 0:128, sl], cb[:, sl])
                nc.sync.dma_start(Out[b, 128:256, sl], a_t[:])
                nc.sync.dma_start(Out[b, 256:384, sl], cta[:])
                nc.sync.dma_start(Out[b, 384:512, sl], ctb[:])

    nc.compile()
    return nc


def _prep_inputs(C, Q, Cmask, Qmask, w_c, w_q, w_mul, bias):
    """Host-side precompute of the folded factors; returns per-core in_maps."""
    import ml_dtypes

    C = np.asarray(C, dtype=np.float32)
    Q = np.asarray(Q, dtype=np.float32)
    cm = np.asarray(Cmask, dtype=np.float32)
    qm = np.asarray(Qmask, dtype=np.float32)
    w_c = np.asarray(w_c, dtype=np.float32).reshape(D)
    w_q = np.asarray(w_q, dtype=np.float32).reshape(D)
    w_mul = np.asarray(w_mul, dtype=np.float32).reshape(D)

    B = C.shape[0]
    s0 = np.einsum("bdc,d->bc", C, w_c)  # [B, Lc]
    s1 = np.einsum("bdq,d->bq", Q, w_q)  # [B, Lq]
    # h[q] = exp(s1 + NEG*(1-qm)); g[c] = exp(s0 + NEG*(1-cm))
    h = np.exp(np.where(qm > 0, s1, NEG))  # [B, Lq]
    g = np.exp(np.where(cm > 0, s0, NEG))  # [B, Lc]

    Qw = Q * w_mul[None, :, None]
    bf = ml_dtypes.bfloat16

    in_maps = []
    for core in range(N_CORES):
        sl = slice(core * B_LOC, (core + 1) * B_LOC)
        hb = h[sl]  # [4, Lq]
        gb = g[sl]  # [4, Lc]
        # hrep[b, p, qt*128+k] = h[b, qt*128+p]
        hrep = np.repeat(
            hb.reshape(B_LOC, NQT, 128).transpose(0, 2, 1), 128, axis=2
        ).reshape(B_LOC, 128, LQ)
        # qth[b, p, qt*128+dd] = Q[b, dd, qt*128+p] * h[b, qt*128+p]
        Qs = Q[sl] * hb[:, None, :]  # [4, d, Lq]
        qth = Qs.reshape(B_LOC, D, NQT, 128).transpose(0, 3, 2, 1).reshape(B_LOC, 128, LQ)
        gcol = gb.reshape(B_LOC, NCT, 128).transpose(0, 2, 1)  # [4,128,16]
        hcol = hb.reshape(B_LOC, NQT, 128).transpose(0, 2, 1)  # [4,128,4]
        in_maps.append({
            "C": np.ascontiguousarray(C[sl]),
            "Qth": np.ascontiguousarray(qth).astype(bf),
            "Qwbf": np.ascontiguousarray(Qw[sl]).astype(bf),
            "Hrep": np.ascontiguousarray(hrep).astype(bf),
            "Gcolb": np.ascontiguousarray(gcol).astype(bf),
            "Gcolf": np.ascontiguousarray(gcol),
            "Hcolf": np.ascontiguousarray(hcol),
        })
    return in_maps


def kernel(C, Q, Cmask, Qmask, w_c, w_q, w_mul, bias):
    from concourse.bass_utils import run_bass_kernel_spmd

    if "nc" not in _NC_CACHE:
        _NC_CACHE["nc"] = _build_bass()
    nc = _NC_CACHE["nc"]

    in_maps = _prep_inputs(C, Q, Cmask, Qmask, w_c, w_q, w_mul, bias)
    res = run_bass_kernel_spmd(nc, in_maps, list(range(N_CORES)))
    out = np.concatenate(
        [res.results[i]["out"] for i in range(N_CORES)], axis=0
    ).astype(np.float32)
    return out


# revision 26
# speedup vs baseline: 1.0273x; 1.0012x over previous
"""CQAttention (BiDAF-style context-query attention) Trainium2 kernel.

Data-parallel over batch: 32 batches -> 8 cores x 4 batches.

Math (per batch, d=128, Lc=2048, Lq=512):
  S = s0[c] + s1[q] + s2[c,q] + bias,  s2 = (Ct*w_mul) @ Qt^T
  S1 = softmax_q(S + NEG*(1-qm));  S2 = softmax_c(S + NEG*(1-cm))
  A  = S1 @ Qt;  Bm = S1 @ (S2^T @ Ct)
  out = [Ct; A; Ct*A; Ct*Bm]^T  -> [4d, Lc]

Device algebra: s0/bias cancel inside softmax_q, s1/bias cancel inside
softmax_c, so both exp passes are the *plain* exp(s2) in the two layouts,
and the per-row/col factors h[q]=exp(s1+qneg), g[c]=exp(s0+cneg) (host
precomputed) are folded multiplicatively into the matmul weight operands:
  rs[c]   = sum_q h[q] X1[q,c]          (lhsT = h replicated)
  An[d,c] = sum_q (Qt*h)[q,d] X1[q,c]
  cs[q]   = sum_c g[c] X2[c,q]          (lhsT = g column)
  NU[d,q] = sum_c (Ct*g)[c,d] X2[c,q]
  Uch[q,d]= NU^T * h[q]/cs[q]
  Bn[d,c] = sum_q Uch[q,d] X1[q,c]
  A = An/rs, Bm = Bn/rs  (PSUM/PSUM divide on DVE)
"""

import sys

sys.path.insert(0, "/opt/trn_rl_repo")

import numpy as np
from contextlib import ExitStack

NEG = -1e30
N_CORES = 8
B_LOC = 4  # batches per core
D = 128
LC = 2048
LQ = 512
NQT = LQ // 128  # 4 q tiles
NCT = LC // 128  # 16 c tiles
NCC = LC // 512  # 4 c chunks
NCG = NCT // 4  # 4 c-tile groups of 4

_NC_CACHE = {}


def _build_bass():
    import concourse.bass as bass
    import concourse.bacc as bacc
    import concourse.tile as tile
    from concourse import mybir, masks

    f32 = mybir.dt.float32
    bf16 = mybir.dt.bfloat16
    Exp = mybir.ActivationFunctionType.Exp
    Alu = mybir.AluOpType

    nc = bacc.Bacc("TRN2", target_bir_lowering=False, debug=False)

    Cin = nc.dram_tensor("C", [B_LOC, D, LC], f32, kind="ExternalInput").ap()
    Qth_in = nc.dram_tensor("Qth", [B_LOC, 128, LQ], bf16, kind="ExternalInput").ap()
    Qwbf_in = nc.dram_tensor("Qwbf", [B_LOC, D, LQ], bf16, kind="ExternalInput").ap()
    Hrep_in = nc.dram_tensor("Hrep", [B_LOC, 128, LQ], bf16, kind="ExternalInput").ap()
    Gcolb_in = nc.dram_tensor("Gcolb", [B_LOC, 128, NCT], bf16, kind="ExternalInput").ap()
    Gcolf_in = nc.dram_tensor("Gcolf", [B_LOC, 128, NCT], f32, kind="ExternalInput").ap()
    Hcolf_in = nc.dram_tensor("Hcolf", [B_LOC, 128, NQT], f32, kind="ExternalInput").ap()
    Out = nc.dram_tensor("out", [B_LOC, 4 * D, LC], f32, kind="ExternalOutput").ap()
    CsScratch = nc.dram_tensor("cs_scratch", [B_LOC, LQ], f32).ap()

    with tile.TileContext(nc) as tc, ExitStack() as ctx:
        cpool = ctx.enter_context(tc.tile_pool(name="const", bufs=1))
        inp = ctx.enter_context(tc.tile_pool(name="inp", bufs=2))
        work = ctx.enter_context(tc.tile_pool(name="work", bufs=2))
        epool = ctx.enter_context(tc.tile_pool(name="epool", bufs=10))
        opool = ctx.enter_context(tc.tile_pool(name="ostg", bufs=8))
        ctgpool = ctx.enter_context(tc.tile_pool(name="ctgp", bufs=6))
        ppw = ctx.enter_context(tc.tile_pool(name="ppw", bufs=3, space="PSUM"))
        pps = ctx.enter_context(tc.tile_pool(name="pps", bufs=2, space="PSUM"))

        ident = cpool.tile([128, 128], bf16, tag="ident")
        masks.make_identity(nc, ident[:])

        for b in range(B_LOC):
            # ---- inputs (small matmul operands first) ----
            qwbf = inp.tile([128, LQ], bf16, tag="qwbf")
            nc.sync.dma_start(qwbf[:], Qwbf_in[b])
            qth = inp.tile([128, LQ], bf16, tag="qth")
            nc.sync.dma_start(qth[:], Qth_in[b])
            cb = inp.tile([128, LC], f32, tag="cb")
            for cc in range(NCC):
                nc.sync.dma_start(
                    cb[:, cc * 512:(cc + 1) * 512],
                    Cin[b][:, cc * 512:(cc + 1) * 512])
            hrep = inp.tile([128, LQ], bf16, tag="hrep")
            nc.sync.dma_start(hrep[:], Hrep_in[b])
            gcolb = inp.tile([128, NCT], bf16, tag="gcolb")
            nc.sync.dma_start(gcolb[:], Gcolb_in[b])
            gcolf = inp.tile([128, NCT], f32, tag="gcolf")
            nc.sync.dma_start(gcolf[:], Gcolf_in[b])
            hcolf = inp.tile([128, NQT], f32, tag="hcolf")
            nc.sync.dma_start(hcolf[:], Hcolf_in[b])

            # warm small DMA'd tensors through DVE so downstream DVE ops
            # carry same-engine deps only (codegen sync-wait slot limits)
            wgcolf = work.tile([128, NCT], f32, tag="wgcolf")
            nc.vector.tensor_copy(wgcolf[:], gcolf[:])
            whcolf = work.tile([128, NQT], f32, tag="whcolf")
            nc.vector.tensor_copy(whcolf[:], hcolf[:])

            # bf16 cast of C on gpsimd (otherwise idle)
            cbf = work.tile([128, LC], bf16, tag="cbf")
            for cc in range(NCC):
                nc.gpsimd.tensor_copy(
                    cbf[:, cc * 512:(cc + 1) * 512],
                    cb[:, cc * 512:(cc + 1) * 512])

            # ---- pass 1: X1[q,c] = exp(s2^T), 4 q-tiles of [128, LC] ----
            e1 = []
            for qt in range(NQT):
                e = epool.tile([128, LC], bf16, tag="e1")
                for h in range(2):
                    ps = ppw.tile([128, LC // 2], f32, tag="wide")
                    for cc in range(2):
                        c0 = (h * 2 + cc) * 512
                        nc.tensor.matmul(
                            ps[:, cc * 512:(cc + 1) * 512],
                            qwbf[:, qt * 128:(qt + 1) * 128],
                            cbf[:, c0:c0 + 512],
                            start=True, stop=True,
                        )
                    nc.scalar.activation(
                        e[:, h * 1024:(h + 1) * 1024], ps[:], Exp)
                e1.append(e)

            # ---- pass 2: X2[c,q] = exp(s2), 4 groups of 4 c-tiles ----
            e2 = []
            for cg in range(NCG):
                e = epool.tile([128, LC], bf16, tag="e2")
                for h in range(2):
                    ps = ppw.tile([128, LC // 2], f32, tag="wide")
                    for j in range(2):
                        ct = cg * 4 + h * 2 + j
                        nc.tensor.matmul(
                            ps[:, j * 512:(j + 1) * 512],
                            cbf[:, ct * 128:(ct + 1) * 128],
                            qwbf[:],
                            start=True, stop=True,
                        )
                    nc.scalar.activation(
                        e[:, h * 1024:(h + 1) * 1024], ps[:], Exp)
                e2.append(e)

            # ---- Ct*g tiles: transpose C then scale by g per c-tile ----
            ctg = []
            for cg in range(NCG):
                ps = pps.tile([128, 512], bf16, tag="sm")
                for j in range(4):
                    ct = cg * 4 + j
                    nc.tensor.transpose(
                        ps[:, j * 128:(j + 1) * 128],
                        cbf[:, ct * 128:(ct + 1) * 128],
                        ident[:],
                    )
                t = ctgpool.tile([128, 512], bf16, tag="ctg")
                for j in range(4):
                    ct = cg * 4 + j
                    nc.vector.tensor_scalar_mul(
                        t[:, j * 128:(j + 1) * 128],
                        ps[:, j * 128:(j + 1) * 128],
                        wgcolf[:, ct:ct + 1],
                    )
                ctg.append(t)

            # ---- cs[q] = sum_c g[c] X2[c,q]  (M=1 reduce) ----
            ps_cs = pps.tile([1, 512], f32, tag="sm")
            for cg in range(NCG):
                for j in range(4):
                    ct = cg * 4 + j
                    nc.tensor.matmul(
                        ps_cs[:],
                        gcolb[:, ct:ct + 1],
                        e2[cg][:, j * 512:(j + 1) * 512],
                        start=(ct == 0), stop=(ct == NCT - 1),
                    )
            # copy cs row to SBUF, scatter [1,512] -> [128,4], hc = h/cs
            cs_row = work.tile([1, 512], f32, tag="csrow")
            nc.vector.tensor_copy(cs_row[:], ps_cs[:])
            nc.sync.dma_start(CsScratch[b], cs_row[0:1, :])
            cs_col = work.tile([128, NQT], f32, tag="cscol")
            nc.sync.dma_start(
                cs_col[:], CsScratch[b].rearrange("(j p) -> p j", j=NQT, p=128)
            )
            csr = work.tile([128, NQT], f32, tag="csr")
            nc.vector.reciprocal(csr[:], cs_col[:])
            hc = work.tile([128, NQT], f32, tag="hc")
            nc.vector.tensor_mul(hc[:], csr[:], whcolf[:])

            # ---- NU[d,q] = sum_c (Ct*g)[c,d] X2[c,q] ----
            ps_ut = pps.tile([128, 512], f32, tag="sm")
            for cg in range(NCG):
                for j in range(4):
                    ct = cg * 4 + j
                    nc.tensor.matmul(
                        ps_ut[:],
                        ctg[cg][:, j * 128:(j + 1) * 128],
                        e2[cg][:, j * 512:(j + 1) * 512],
                        start=(ct == 0), stop=(ct == NCT - 1),
                    )
            utb = work.tile([128, 512], bf16, tag="utb")
            nc.vector.tensor_copy(utb[:], ps_ut[:])

            # ---- Uch[q,d] = NU^T * h/cs ----
            ps_u2 = pps.tile([128, 512], bf16, tag="sm")
            for qt in range(NQT):
                nc.tensor.transpose(
                    ps_u2[:, qt * 128:(qt + 1) * 128],
                    utb[:, qt * 128:(qt + 1) * 128],
                    ident[:],
                )
            uch = work.tile([128, 512], bf16, tag="uch")
            for qt in range(NQT):
                nc.vector.tensor_scalar_mul(
                    uch[:, qt * 128:(qt + 1) * 128],
                    ps_u2[:, qt * 128:(qt + 1) * 128],
                    hc[:, qt:qt + 1],
                )

            # ---- rs[c] = sum_q h[q] X1[q,c] (replicated rows); transient
            # psum per c-chunk, immediately reciprocal'd into SBUF ----
            rrec = work.tile([128, LC], f32, tag="rrec")
            for cc in range(NCC):
                ps_rs = pps.tile([128, 512], f32, tag="sm")
                for qt in range(NQT):
                    nc.tensor.matmul(
                        ps_rs[:],
                        hrep[:, qt * 128:(qt + 1) * 128],
                        e1[qt][:, cc * 512:(cc + 1) * 512],
                        start=(qt == 0), stop=(qt == NQT - 1),
                    )
                nc.vector.reciprocal(rrec[:, cc * 512:(cc + 1) * 512], ps_rs[:])

            # ---- An, Bn per c-chunk; outputs ----
            for cc in range(NCC):
                sl = slice(cc * 512, (cc + 1) * 512)

                ps_an = pps.tile([128, 512], f32, tag="sm")
                for qt in range(NQT):
                    nc.tensor.matmul(
                        ps_an[:],
                        qth[:, qt * 128:(qt + 1) * 128],
                        e1[qt][:, sl],
                        start=(qt == 0), stop=(qt == NQT - 1),
                    )
                a_t = opool.tile([128, 512], f32, tag="a")
                nc.vector.scalar_tensor_tensor(
                    a_t[:], ps_an[:], 0.0, rrec[:, sl],
                    op0=Alu.bypass, op1=Alu.mult,
                )

                ps_bn = pps.tile([128, 512], f32, tag="sm")
                for qt in range(NQT):
                    nc.tensor.matmul(
                        ps_bn[:],
                        uch[:, qt * 128:(qt + 1) * 128],
                        e1[qt][:, sl],
                        start=(qt == 0), stop=(qt == NQT - 1),
                    )
                bm_t = opool.tile([128, 512], f32, tag="bm")
                nc.vector.scalar_tensor_tensor(
                    bm_t[:], ps_bn[:], 0.0, rrec[:, sl],
                    op0=Alu.bypass, op1=Alu.mult,
                )

                cta = opool.tile([128, 512], f32, tag="cta")
                nc.gpsimd.tensor_mul(cta[:], cb[:, sl], a_t[:])
                ctb = opool.tile([128, 512], f32, tag="ctb")
                nc.gpsimd.tensor_mul(ctb[:], cb[:, sl], bm_t[:])

                nc.sync.dma_start(Out[b, 0:128, sl], cb[:, sl])
                nc.sync.dma_start(Out[b, 128:256, sl], a_t[:])
                nc.sync.dma_start(Out[b, 256:384, sl], cta[:])
                nc.sync.dma_start(Out[b, 384:512, sl], ctb[:])

    nc.compile()
    return nc


def _prep_inputs(C, Q, Cmask, Qmask, w_c, w_q, w_mul, bias):
    """Host-side precompute of the folded factors; returns per-core in_maps."""
    import ml_dtypes

    C = np.asarray(C, dtype=np.float32)
    Q = np.asarray(Q, dtype=np.float32)
    cm = np.asarray(Cmask, dtype=np.float32)
    qm = np.asarray(Qmask, dtype=np.float32)
    w_c = np.asarray(w_c, dtype=np.float32).reshape(D)
    w_q = np.asarray(w_q, dtype=np.float32).reshape(D)
    w_mul = np.asarray(w_mul, dtype=np.float32).reshape(D)

    B = C.shape[0]
    s0 = np.einsum("bdc,d->bc", C, w_c)  # [B, Lc]
    s1 = np.einsum("bdq,d->bq", Q, w_q)  # [B, Lq]
    # h[q] = exp(s1 + NEG*(1-qm)); g[c] = exp(s0 + NEG*(1-cm))
    h = np.exp(np.where(qm > 0, s1, NEG))  # [B, Lq]
    g = np.exp(np.where(cm > 0, s0, NEG))  # [B, Lc]

    Qw = Q * w_mul[None, :, None]
    bf = ml_dtypes.bfloat16

    in_maps = []
    for core in range(N_CORES):
        sl = slice(core * B_LOC, (core + 1) * B_LOC)
        hb = h[sl]  # [4, Lq]
        gb = g[sl]  # [4, Lc]
        # hrep[b, p, qt*128+k] = h[b, qt*128+p]
        hrep = np.repeat(
            hb.reshape(B_LOC, NQT, 128).transpose(0, 2, 1), 128, axis=2
        ).reshape(B_LOC, 128, LQ)
        # qth[b, p, qt*128+dd] = Q[b, dd, qt*128+p] * h[b, qt*128+p]
        Qs = Q[sl] * hb[:, None, :]  # [4, d, Lq]
        qth = Qs.reshape(B_LOC, D, NQT, 128).transpose(0, 3, 2, 1).reshape(B_LOC, 128, LQ)
        gcol = gb.reshape(B_LOC, NCT, 128).transpose(0, 2, 1)  # [4,128,16]
        hcol = hb.reshape(B_LOC, NQT, 128).transpose(0, 2, 1)  # [4,128,4]
        in_maps.append({
            "C": np.ascontiguousarray(C[sl]),
            "Qth": np.ascontiguousarray(qth).astype(bf),
            "Qwbf": np.ascontiguousarray(Qw[sl]).astype(bf),
            "Hrep": np.ascontiguousarray(hrep).astype(bf),
            "Gcolb": np.ascontiguousarray(gcol).astype(bf),
            "Gcolf": np.ascontiguousarray(gcol),
            "Hcolf": np.ascontiguousarray(hcol),
        })
    return in_maps


def kernel(C, Q, Cmask, Qmask, w_c, w_q, w_mul, bias):
    from concourse.bass_utils import run_bass_kernel_spmd

    if "nc" not in _NC_CACHE:
        _NC_CACHE["nc"] = _build_bass()
    nc = _NC_CACHE["nc"]

    in_maps = _prep_inputs(C, Q, Cmask, Qmask, w_c, w_q, w_mul, bias)
    res = run_bass_kernel_spmd(nc, in_maps, list(range(N_CORES)))
    out = np.concatenate(
        [res.results[i]["out"] for i in range(N_CORES)], axis=0
    ).astype(np.float32)
    return out
